# revision 6
# baseline (speedup 1.0000x reference)
"""KPConv Bass/Trainium2 kernel.

out[m,d] = sum_k ( sum_h infl[m,h,k] * s_feats[idx[m,h],:] ) @ W[k]
infl[m,h,k] = relu(1 - |s_pts[idx[m,h]] - q_pts[m] - kp[k]| / SIGMA)

Sharding: query points M=50000 split 8 ways (6250/core, padded to 6272 =
49 blocks x 128 points). Support table / weights replicated per core.

Key layout choices vs the naive version:
  - One fused support table [N, 136] bf16: cols 0:128 = s_feats (bf16),
    cols 128:134 = s_pts (f32 bit-packed into bf16 pairs). Each indirect
    DMA gathers feats AND coords together (halves the DMA count vs
    separate feat/coord gathers; the ~1.1us/instr SWDGE fixed cost is
    the kernel's bottleneck - HW indirect DMA consumes exactly one
    index per partition, so 32 gathers per block is the floor; the
    batched dma_gather instruction needs GPSIMD ucode libraries that
    this environment does not provide).
  - Block/query metadata (indices, query coords) loaded once up front.
  - Matmuls in bf16 (4x faster PE streaming than f32), accumulate f32.

Per-core dataflow, per block of 128 query points (= 32 tiles of 4
points x 32 neighbors = 128 edges each):
  1. 32 indirect-DMA gathers of fused rows -> nfc [128, 32*136].
  2. influence on DVE/ACT (f32): delta, (delta-kp)^2, segmented reduce,
     sqrt, relu affine -> infl [128, 32*15]; block-diag mask -> bd bf16.
  3. step A on PE: per tile t, matmul(lhsT=feats_t [128e,128c] bf16,
     rhs=block-diag influence [128e, 60] bf16) -> PSUM wfT [128c, m*15+k].
  4. step B on PE: per k, matmul(lhsT=wfT[:, k::15] [c,m] bf16,
     rhs=W[k] [c,d] bf16) accumulating over k -> PSUM [128m, 128d] ->
     SBUF -> DRAM.
"""

import sys

sys.path.insert(0, "/opt/trn_rl_repo")

import numpy as np

# ---------------------------------------------------------------- constants
N_CORES = 8
M_TOTAL = 50000
N_SUP = 50000
H = 32
C = 128
K = 15
SIGMA = 2.0

M_CORE = M_TOTAL // N_CORES          # 6250
P = 128                              # partitions / points per block
NB = (M_CORE + P - 1) // P           # 49 blocks
M_PAD = NB * P                       # 6272
G = 4                                # points per step-A matmul tile
NT = P // G                          # 32 tiles per block
TW = C + 8                           # fused table row: 128 feat + 6 coord + 2 pad

_compiled = None


def _build_bass(nb=NB, n_sup=N_SUP, compile=True):
    """Build + compile the per-core SPMD Bass program."""
    from contextlib import ExitStack

    import concourse.bacc as bacc
    import concourse.mybir as mybir
    import concourse.tile as tile
    from concourse import bass

    f32 = mybir.dt.float32
    bf16 = mybir.dt.bfloat16
    i32 = mybir.dt.int32
    NB = nb
    N_SUP_ = n_sup

    nc = bacc.Bacc(
        "TRN2",
        target_bir_lowering=False,
        debug=False,
        enable_asserts=False,
        num_devices=N_CORES,
    )

    q_all_d = nc.dram_tensor("q_all", (P, NB * NT * 3), f32, kind="ExternalInput")
    inds_d = nc.dram_tensor("inds_all", (P, NB * NT), i32, kind="ExternalInput")
    table_d = nc.dram_tensor("table", (N_SUP_, TW), bf16, kind="ExternalInput")
    w_d = nc.dram_tensor("weights", (K, C, C), bf16, kind="ExternalInput")
    kp_d = nc.dram_tensor("kp_rep", (P, K * 3), f32, kind="ExternalInput")
    mask_d = nc.dram_tensor("mask60", (P, G * K), f32, kind="ExternalInput")
    out_d = nc.dram_tensor("out", (NB, P, C), f32, kind="ExternalOutput")

    sub = mybir.AluOpType.subtract
    mult = mybir.AluOpType.mult

    with tile.TileContext(nc) as tc, ExitStack() as ctx:
        const = ctx.enter_context(tc.tile_pool(name="const", bufs=1))
        io = ctx.enter_context(tc.tile_pool(name="io", bufs=2))
        mid = ctx.enter_context(tc.tile_pool(name="mid", bufs=2))
        psa = ctx.enter_context(tc.tile_pool(name="psa", bufs=1, space="PSUM"))
        psb = ctx.enter_context(tc.tile_pool(name="psb", bufs=2, space="PSUM"))

        # constants: weights as [c, (k d)], kernel points, block-diag mask,
        # all per-block query coords + neighbor indices
        w_sb = const.tile([P, K, C], bf16)
        nc.sync.dma_start(w_sb[:], w_d.ap().rearrange("k c d -> c k d"))
        kp_sb = const.tile([P, K * 3], f32)
        nc.sync.dma_start(kp_sb[:], kp_d.ap())
        mask_sb = const.tile([P, G * K], f32)
        nc.sync.dma_start(mask_sb[:], mask_d.ap())
        q_sb = const.tile([P, NB * NT * 3], f32)
        nc.sync.dma_start(q_sb[:], q_all_d.ap())
        inds_sb = const.tile([P, NB * NT], i32)
        nc.sync.dma_start(inds_sb[:], inds_d.ap())

        for B in range(NB):
            # gather fused rows tile by tile (HW indirect DMA consumes
            # exactly one index per partition): row inds[p, B*NT+t] ->
            # nfc[p, t*TW:(t+1)*TW]
            nfc = io.tile([P, NT * TW], bf16, tag="nfc")
            for t in range(NT):
                nc.gpsimd.indirect_dma_start(
                    out=nfc[:, t * TW : (t + 1) * TW],
                    out_offset=None,
                    in_=table_d.ap(),
                    in_offset=bass.IndirectOffsetOnAxis(
                        ap=inds_sb[:, B * NT + t : B * NT + t + 1], axis=0
                    ),
                )

            # influence (f32 path on DVE/ACT)
            sg = (
                nfc[:]
                .rearrange("p (t x) -> p t x", x=TW)[:, :, C : C + 6]
                .bitcast(f32)
            )  # [P, NT, 3] gathered support coords
            qb = q_sb[:, B * NT * 3 : (B + 1) * NT * 3].rearrange(
                "p (t j) -> p t j", j=3
            )
            delta = mid.tile([P, NT * 3], f32, tag="delta")
            nc.vector.tensor_tensor(
                delta[:].rearrange("p (t j) -> p t j", j=3), sg, qb, op=sub
            )

            diff = mid.tile([P, NT * K * 3], f32, tag="diff")
            nc.vector.tensor_tensor(
                diff[:].rearrange("p (t k j) -> p t k j", k=K, j=3),
                delta[:].rearrange("p (t j) -> p t j", j=3)
                .unsqueeze(2)
                .broadcast_to([P, NT, K, 3]),
                kp_sb[:].rearrange("p (k j) -> p k j", j=3)
                .unsqueeze(1)
                .broadcast_to([P, NT, K, 3]),
                op=sub,
            )
            sq = mid.tile([P, NT * K * 3], f32, tag="sq")
            nc.vector.tensor_tensor(sq[:], diff[:], diff[:], op=mult)
            d2 = mid.tile([P, NT * K], f32, tag="d2")
            nc.vector.reduce_sum(
                out=d2[:],
                in_=sq[:].rearrange("p (tk j) -> p tk j", j=3),
                axis=mybir.AxisListType.X,
            )
            dd = mid.tile([P, NT * K], f32, tag="dd")
            nc.scalar.sqrt(dd[:], d2[:])
            infl = mid.tile([P, NT * K], f32, tag="infl")
            nc.scalar.activation(
                infl[:],
                dd[:],
                mybir.ActivationFunctionType.Relu,
                bias=1.0,
                scale=-1.0 / SIGMA,
            )

            # block-diagonal influence [p, t*60 + g*15 + k], cast to bf16
            bd = mid.tile([P, NT * G * K], bf16, tag="bd")
            nc.vector.tensor_tensor(
                bd[:].rearrange("p (t g k) -> p t g k", g=G, k=K),
                infl[:].rearrange("p (t k) -> p t k", k=K)
                .unsqueeze(2)
                .broadcast_to([P, NT, G, K]),
                mask_sb[:].rearrange("p (g k) -> p g k", k=K)
                .unsqueeze(1)
                .broadcast_to([P, NT, G, K]),
                op=mult,
            )

            # step A: 32 bf16 matmuls -> wfT[c, m*15+k] in 4 PSUM banks
            pa = [
                psa.tile([P, 8 * G * K], f32, tag=f"psA{q}", name=f"psA{q}")
                for q in range(4)
            ]
            for t in range(NT):
                nc.tensor.matmul(
                    pa[t // 8][:, (t % 8) * (G * K) : (t % 8 + 1) * (G * K)],
                    lhsT=nfc[:, t * TW : t * TW + C],
                    rhs=bd[:, t * (G * K) : (t + 1) * (G * K)],
                    start=True,
                    stop=True,
                )
            wfT = mid.tile([P, P * K], bf16, tag="wfT")
            for q in range(4):
                nc.scalar.copy(wfT[:, q * 480 : (q + 1) * 480], pa[q][:])

            # step B: accumulate over k (bf16 operands, f32 PSUM)
            outp = psb.tile([P, C], f32, tag="outp")
            wview = wfT[:].rearrange("p (m k) -> p k m", k=K)
            for k in range(K):
                nc.tensor.matmul(
                    outp[:],
                    lhsT=wview[:, k, :],
                    rhs=w_sb[:, k, :],
                    start=(k == 0),
                    stop=(k == K - 1),
                )
            osb = mid.tile([P, C], f32, tag="osb")
            nc.scalar.copy(osb[:], outp[:])
            nc.sync.dma_start(out_d.ap()[B], osb[:])

    if compile:
        nc.compile()
    return nc


def _host_prep(q_pts, s_pts, s_feats, neighb_inds, weights, kernel_points):
    """Shard + lay out inputs for the 8 cores."""
    import ml_dtypes

    bf16 = ml_dtypes.bfloat16

    q_pts = np.asarray(q_pts, np.float32)
    s_pts = np.asarray(s_pts, np.float32)
    s_feats = np.asarray(s_feats, np.float32)
    neighb_inds = np.asarray(neighb_inds, np.int32)
    weights = np.asarray(weights, np.float32).astype(bf16)
    kernel_points = np.asarray(kernel_points, np.float32)

    # fused support table: bf16 feats + f32 coords bit-packed as bf16 pairs
    table = np.zeros((N_SUP, TW), dtype=bf16)
    table[:, :C] = s_feats.astype(bf16)
    table_u16 = table.view(np.uint16)
    table_u16[:, C : C + 6] = s_pts.view(np.uint16).reshape(N_SUP, 6)

    kp_rep = np.broadcast_to(
        kernel_points.reshape(1, K * 3), (P, K * 3)
    ).copy()
    mask60 = (
        (np.arange(G * K)[None, :] // K) == (np.arange(P)[:, None] // H)
    ).astype(np.float32)

    in_maps = []
    for i in range(N_CORES):
        sl = slice(i * M_CORE, (i + 1) * M_CORE)
        q = np.zeros((M_PAD, 3), np.float32)
        q[:M_CORE] = q_pts[sl]
        idx = np.zeros((M_PAD, H), np.int32)
        idx[:M_CORE] = neighb_inds[sl]

        # inds_all[g*32+h, B*NT + t] = idx[B*128 + 4t + g, h]
        a = idx.reshape(NB, NT, G, H)            # [B, t, g, h]
        inds_all = np.ascontiguousarray(
            a.transpose(2, 3, 0, 1)              # [g, h, B, t]
        ).reshape(P, NB * NT)

        # q_all[g*32+h, B*96 + 3t+j] = q[B*128 + 4t + g, j]
        b = q.reshape(NB, NT, G, 3)              # [B, t, g, j]
        b = b.transpose(2, 0, 1, 3)              # [g, B, t, j]
        q_all = np.broadcast_to(
            b.reshape(G, 1, NB, NT * 3), (G, H, NB, NT * 3)
        ).reshape(P, NB * NT * 3)

        in_maps.append(
            {
                "q_all": np.ascontiguousarray(q_all),
                "inds_all": inds_all,
                "table": table,
                "weights": weights,
                "kp_rep": kp_rep,
                "mask60": mask60,
            }
        )
    return in_maps


def kernel(q_pts, s_pts, s_feats, neighb_inds, weights, kernel_points):
    global _compiled
    if _compiled is None:
        _compiled = _build_bass()
    nc = _compiled

    from concourse.bass_utils import run_bass_kernel_spmd

    in_maps = _host_prep(
        q_pts, s_pts, s_feats, neighb_inds, weights, kernel_points
    )
    res = run_bass_kernel_spmd(nc, in_maps, core_ids=list(range(N_CORES)))
    out = np.concatenate(
        [r["out"].reshape(M_PAD, C)[:M_CORE] for r in res.results], axis=0
    )
    return out.astype(np.float32)


if __name__ == "__main__":
    rng = np.random.default_rng(0)
    ins = {
        "q_pts": rng.standard_normal((M_TOTAL, 3)).astype(np.float32),
        "s_pts": rng.standard_normal((N_SUP, 3)).astype(np.float32),
        "s_feats": rng.standard_normal((N_SUP, C)).astype(np.float32),
        "neighb_inds": rng.integers(0, N_SUP, (M_TOTAL, H)).astype(np.int32),
        "weights": (rng.standard_normal((K, C, C)) * 0.05).astype(np.float32),
        "kernel_points": rng.standard_normal((K, 3)).astype(np.float32),
    }
    out = kernel(**ins)
    print(out.shape, out.dtype)


# revision 8
# speedup vs baseline: 1.1886x; 1.1886x over previous
"""KPConv Bass/Trainium2 kernel.

out[m,d] = sum_k ( sum_h infl[m,h,k] * s_feats[idx[m,h],:] ) @ W[k]
infl[m,h,k] = relu(1 - |s_pts[idx[m,h]] - q_pts[m] - kp[k]| / SIGMA)

Sharding: query points M=50000 split 8 ways (6250/core, padded to 6272 =
49 blocks x 128 points). Support table / weights replicated per core.

Key layout choices vs the naive version:
  - One fused support table [N, 136] bf16: cols 0:128 = s_feats (bf16),
    cols 128:134 = s_pts (f32 bit-packed into bf16 pairs). Each indirect
    DMA gathers feats AND coords together (halves the DMA count vs
    separate feat/coord gathers; the ~1.1us/instr SWDGE fixed cost is
    the kernel's bottleneck - HW indirect DMA consumes exactly one
    index per partition, so 32 gathers per block is the floor; the
    batched dma_gather instruction needs GPSIMD ucode libraries that
    this environment does not provide).
  - Block/query metadata (indices, query coords) loaded once up front.
  - Matmuls in bf16 (4x faster PE streaming than f32), accumulate f32.

Per-core dataflow, per block of 128 query points (= 32 tiles of 4
points x 32 neighbors = 128 edges each):
  1. 32 indirect-DMA gathers of fused rows -> nfc [128, 32*136].
  2. influence on DVE/ACT (f32): delta, (delta-kp)^2, segmented reduce,
     sqrt, relu affine -> infl [128, 32*15]; block-diag mask -> bd bf16.
  3. step A on PE: per tile t, matmul(lhsT=feats_t [128e,128c] bf16,
     rhs=block-diag influence [128e, 60] bf16) -> PSUM wfT [128c, m*15+k].
  4. step B on PE: per k, matmul(lhsT=wfT[:, k::15] [c,m] bf16,
     rhs=W[k] [c,d] bf16) accumulating over k -> PSUM [128m, 128d] ->
     SBUF -> DRAM.
"""

import sys

sys.path.insert(0, "/opt/trn_rl_repo")

import numpy as np

# ---------------------------------------------------------------- constants
N_CORES = 8
M_TOTAL = 50000
N_SUP = 50000
H = 32
C = 128
K = 15
SIGMA = 2.0

M_CORE = M_TOTAL // N_CORES          # 6250
P = 128                              # partitions / points per block
NB = (M_CORE + P - 1) // P           # 49 blocks
M_PAD = NB * P                       # 6272
G = 4                                # points per step-A matmul tile
NT = P // G                          # 32 tiles per block
TW = C + 8                           # fused table row: 128 feat + 6 coord + 2 pad

_compiled = None


def _build_bass(nb=NB, n_sup=N_SUP, compile=True):
    """Build + compile the per-core SPMD Bass program."""
    from contextlib import ExitStack

    import concourse.bacc as bacc
    import concourse.mybir as mybir
    import concourse.tile as tile
    from concourse import bass

    f32 = mybir.dt.float32
    bf16 = mybir.dt.bfloat16
    i32 = mybir.dt.int32
    NB = nb
    N_SUP_ = n_sup

    nc = bacc.Bacc(
        "TRN2",
        target_bir_lowering=False,
        debug=False,
        enable_asserts=False,
        num_devices=N_CORES,
    )

    q_all_d = nc.dram_tensor("q_all", (P, NB * NT * 3), f32, kind="ExternalInput")
    inds_d = nc.dram_tensor("inds_all", (P, NB * NT), i32, kind="ExternalInput")
    table_d = nc.dram_tensor("table", (N_SUP_, TW), bf16, kind="ExternalInput")
    w_d = nc.dram_tensor("weights", (K, C, C), bf16, kind="ExternalInput")
    kp_d = nc.dram_tensor("kp_rep", (P, K * 3), f32, kind="ExternalInput")
    mask_d = nc.dram_tensor("mask60", (P, G * K), f32, kind="ExternalInput")
    out_d = nc.dram_tensor("out", (NB, P, C), f32, kind="ExternalOutput")

    sub = mybir.AluOpType.subtract
    mult = mybir.AluOpType.mult

    with tile.TileContext(nc) as tc, ExitStack() as ctx:
        const = ctx.enter_context(tc.tile_pool(name="const", bufs=1))
        io = ctx.enter_context(tc.tile_pool(name="io", bufs=3))
        mid = ctx.enter_context(tc.tile_pool(name="mid", bufs=2))
        psa = ctx.enter_context(tc.tile_pool(name="psa", bufs=1, space="PSUM"))
        psb = ctx.enter_context(tc.tile_pool(name="psb", bufs=2, space="PSUM"))

        # constants: weights as [c, (k d)], kernel points, block-diag mask,
        # all per-block query coords + neighbor indices
        w_sb = const.tile([P, K, C], bf16)
        nc.sync.dma_start(w_sb[:], w_d.ap().rearrange("k c d -> c k d"))
        kp_sb = const.tile([P, K * 3], f32)
        nc.sync.dma_start(kp_sb[:], kp_d.ap())
        mask_sb = const.tile([P, G * K], f32)
        nc.sync.dma_start(mask_sb[:], mask_d.ap())
        q_sb = const.tile([P, NB * NT * 3], f32)
        nc.sync.dma_start(q_sb[:], q_all_d.ap())
        inds_sb = const.tile([P, NB * NT], i32)
        nc.sync.dma_start(inds_sb[:], inds_d.ap())

        for B in range(NB):
            # gather fused rows tile by tile (HW indirect DMA consumes
            # exactly one index per partition): row inds[p, B*NT+t] ->
            # nfc[p, t*TW:(t+1)*TW]
            nfc = io.tile([P, NT * TW], bf16, tag="nfc")
            for t in range(NT):
                nc.gpsimd.indirect_dma_start(
                    out=nfc[:, t * TW : (t + 1) * TW],
                    out_offset=None,
                    in_=table_d.ap(),
                    in_offset=bass.IndirectOffsetOnAxis(
                        ap=inds_sb[:, B * NT + t : B * NT + t + 1], axis=0
                    ),
                    oob_is_err=False,
                )

            # influence (f32 path on DVE/ACT)
            sg = (
                nfc[:]
                .rearrange("p (t x) -> p t x", x=TW)[:, :, C : C + 6]
                .bitcast(f32)
            )  # [P, NT, 3] gathered support coords
            qb = q_sb[:, B * NT * 3 : (B + 1) * NT * 3].rearrange(
                "p (t j) -> p t j", j=3
            )
            delta = mid.tile([P, NT * 3], f32, tag="delta")
            nc.vector.tensor_tensor(
                delta[:].rearrange("p (t j) -> p t j", j=3), sg, qb, op=sub
            )

            diff = mid.tile([P, NT * K * 3], f32, tag="diff")
            nc.vector.tensor_tensor(
                diff[:].rearrange("p (t k j) -> p t k j", k=K, j=3),
                delta[:].rearrange("p (t j) -> p t j", j=3)
                .unsqueeze(2)
                .broadcast_to([P, NT, K, 3]),
                kp_sb[:].rearrange("p (k j) -> p k j", j=3)
                .unsqueeze(1)
                .broadcast_to([P, NT, K, 3]),
                op=sub,
            )
            sq = mid.tile([P, NT * K * 3], f32, tag="sq")
            nc.vector.tensor_tensor(sq[:], diff[:], diff[:], op=mult)
            d2 = mid.tile([P, NT * K], f32, tag="d2")
            nc.vector.reduce_sum(
                out=d2[:],
                in_=sq[:].rearrange("p (tk j) -> p tk j", j=3),
                axis=mybir.AxisListType.X,
            )
            dd = mid.tile([P, NT * K], f32, tag="dd")
            nc.scalar.sqrt(dd[:], d2[:])
            infl = mid.tile([P, NT * K], f32, tag="infl")
            nc.scalar.activation(
                infl[:],
                dd[:],
                mybir.ActivationFunctionType.Relu,
                bias=1.0,
                scale=-1.0 / SIGMA,
            )

            # block-diagonal influence [p, t*60 + g*15 + k], cast to bf16
            bd = mid.tile([P, NT * G * K], bf16, tag="bd")
            nc.vector.tensor_tensor(
                bd[:].rearrange("p (t g k) -> p t g k", g=G, k=K),
                infl[:].rearrange("p (t k) -> p t k", k=K)
                .unsqueeze(2)
                .broadcast_to([P, NT, G, K]),
                mask_sb[:].rearrange("p (g k) -> p g k", k=K)
                .unsqueeze(1)
                .broadcast_to([P, NT, G, K]),
                op=mult,
            )

            # step A: 32 bf16 matmuls -> wfT[c, m*15+k] in 4 PSUM banks
            pa = [
                psa.tile([P, 8 * G * K], f32, tag=f"psA{q}", name=f"psA{q}")
                for q in range(4)
            ]
            for t in range(NT):
                nc.tensor.matmul(
                    pa[t // 8][:, (t % 8) * (G * K) : (t % 8 + 1) * (G * K)],
                    lhsT=nfc[:, t * TW : t * TW + C],
                    rhs=bd[:, t * (G * K) : (t + 1) * (G * K)],
                    start=True,
                    stop=True,
                )
            wfT = mid.tile([P, P * K], bf16, tag="wfT")
            for q in range(4):
                nc.scalar.copy(wfT[:, q * 480 : (q + 1) * 480], pa[q][:])

            # step B: accumulate over k (bf16 operands, f32 PSUM)
            outp = psb.tile([P, C], f32, tag="outp")
            wview = wfT[:].rearrange("p (m k) -> p k m", k=K)
            for k in range(K):
                nc.tensor.matmul(
                    outp[:],
                    lhsT=wview[:, k, :],
                    rhs=w_sb[:, k, :],
                    start=(k == 0),
                    stop=(k == K - 1),
                )
            osb = mid.tile([P, C], f32, tag="osb")
            nc.scalar.copy(osb[:], outp[:])
            nc.sync.dma_start(out_d.ap()[B], osb[:])

    if compile:
        nc.compile()
    return nc


def _host_prep(q_pts, s_pts, s_feats, neighb_inds, weights, kernel_points):
    """Shard + lay out inputs for the 8 cores."""
    import ml_dtypes

    bf16 = ml_dtypes.bfloat16

    q_pts = np.asarray(q_pts, np.float32)
    s_pts = np.asarray(s_pts, np.float32)
    s_feats = np.asarray(s_feats, np.float32)
    neighb_inds = np.asarray(neighb_inds, np.int32)
    weights = np.asarray(weights, np.float32).astype(bf16)
    kernel_points = np.asarray(kernel_points, np.float32)

    # fused support table: bf16 feats + f32 coords bit-packed as bf16 pairs
    table = np.zeros((N_SUP, TW), dtype=bf16)
    table[:, :C] = s_feats.astype(bf16)
    table_u16 = table.view(np.uint16)
    table_u16[:, C : C + 6] = s_pts.view(np.uint16).reshape(N_SUP, 6)

    kp_rep = np.broadcast_to(
        kernel_points.reshape(1, K * 3), (P, K * 3)
    ).copy()
    mask60 = (
        (np.arange(G * K)[None, :] // K) == (np.arange(P)[:, None] // H)
    ).astype(np.float32)

    in_maps = []
    for i in range(N_CORES):
        sl = slice(i * M_CORE, (i + 1) * M_CORE)
        q = np.zeros((M_PAD, 3), np.float32)
        q[:M_CORE] = q_pts[sl]
        idx = np.zeros((M_PAD, H), np.int32)
        idx[:M_CORE] = neighb_inds[sl]

        # inds_all[g*32+h, B*NT + t] = idx[B*128 + 4t + g, h]
        a = idx.reshape(NB, NT, G, H)            # [B, t, g, h]
        inds_all = np.ascontiguousarray(
            a.transpose(2, 3, 0, 1)              # [g, h, B, t]
        ).reshape(P, NB * NT)

        # q_all[g*32+h, B*96 + 3t+j] = q[B*128 + 4t + g, j]
        b = q.reshape(NB, NT, G, 3)              # [B, t, g, j]
        b = b.transpose(2, 0, 1, 3)              # [g, B, t, j]
        q_all = np.broadcast_to(
            b.reshape(G, 1, NB, NT * 3), (G, H, NB, NT * 3)
        ).reshape(P, NB * NT * 3)

        in_maps.append(
            {
                "q_all": np.ascontiguousarray(q_all),
                "inds_all": inds_all,
                "table": table,
                "weights": weights,
                "kp_rep": kp_rep,
                "mask60": mask60,
            }
        )
    return in_maps


def kernel(q_pts, s_pts, s_feats, neighb_inds, weights, kernel_points):
    global _compiled
    if _compiled is None:
        _compiled = _build_bass()
    nc = _compiled

    from concourse.bass_utils import run_bass_kernel_spmd

    in_maps = _host_prep(
        q_pts, s_pts, s_feats, neighb_inds, weights, kernel_points
    )
    res = run_bass_kernel_spmd(nc, in_maps, core_ids=list(range(N_CORES)))
    out = np.concatenate(
        [r["out"].reshape(M_PAD, C)[:M_CORE] for r in res.results], axis=0
    )
    return out.astype(np.float32)


if __name__ == "__main__":
    rng = np.random.default_rng(0)
    ins = {
        "q_pts": rng.standard_normal((M_TOTAL, 3)).astype(np.float32),
        "s_pts": rng.standard_normal((N_SUP, 3)).astype(np.float32),
        "s_feats": rng.standard_normal((N_SUP, C)).astype(np.float32),
        "neighb_inds": rng.integers(0, N_SUP, (M_TOTAL, H)).astype(np.int32),
        "weights": (rng.standard_normal((K, C, C)) * 0.05).astype(np.float32),
        "kernel_points": rng.standard_normal((K, 3)).astype(np.float32),
    }
    out = kernel(**ins)
    print(out.shape, out.dtype)
